# revision 1
# baseline (speedup 1.0000x reference)
import numpy as np

# nn_AttentionGCN: B=8192 nodes, L=32 neighbors, D=128, H=8 heads, 2 layers.
# Sharding: data-parallel over the node batch across 8 NeuronCores (each node's
# attention/projection math is independent); small weight matrices replicated.
B, L, D, H = 8192, 32, 128, 8
NCORES = 8


def _forward_np(node_embeds, neighbor_embeds, node_degrees,
                wq0, bq0, wv0, bv0, wp0, bp0,
                wq1, bq1, wv1, bv1, wp1, bp1):
    """Pure-numpy fallback (host)."""
    def attn(node, neigh, mask, wq, bq, wv, bv, wp, bp, concatenate):
        b, l, d = neigh.shape
        v = neigh @ wv.T + bv
        dv = v.shape[-1] // H
        v = v.reshape(b, l, H, dv).transpose(0, 2, 1, 3)
        q = (node @ wq.T + bq).reshape(b, H, d)
        scores = np.einsum('bhd,bld->bhl', q, neigh) / np.sqrt(np.float32(d))
        scores = np.where(mask[:, None, :], scores, np.float32(-1e9))
        scores = scores - scores.max(axis=-1, keepdims=True)
        e = np.exp(scores)
        p = e / e.sum(axis=-1, keepdims=True)
        av = np.einsum('bhl,bhld->bhd', p, v)
        av = av.reshape(b, H * dv) if concatenate else av.mean(axis=1)
        return node @ wp.T + bp + av

    mask = np.arange(L)[None, :] < node_degrees[:, None]
    neigh = np.where(mask[:, :, None], neighbor_embeds, np.float32(0.0))
    x = attn(node_embeds, neigh, mask, wq0, bq0, wv0, bv0, wp0, bp0, True)
    x = np.maximum(x, np.float32(0.0))
    x = attn(x, neigh, mask, wq1, bq1, wv1, bv1, wp1, bp1, False)
    return x.astype(np.float32)


def _forward_trn(node_embeds, neighbor_embeds, node_degrees, weights):
    """Data-parallel execution on the 8 trn2 NeuronCores via jax pmap."""
    import jax
    import jax.numpy as jnp

    devs = jax.devices()[:NCORES]
    if len(devs) < NCORES:
        raise RuntimeError("need 8 devices")

    def attn(node, neigh, mask, wq, bq, wv, bv, wp, bp, concatenate):
        b = node.shape[0]
        v = neigh @ wv.T + bv
        dv = v.shape[-1] // H
        v = v.reshape(b, L, H, dv).transpose(0, 2, 1, 3)
        q = (node @ wq.T + bq).reshape(b, H, D)
        scores = jnp.einsum('bhd,bld->bhl', q, neigh) / jnp.sqrt(jnp.float32(D))
        scores = jnp.where(mask[:, None, :], scores, jnp.float32(-1e9))
        p = jax.nn.softmax(scores, axis=-1)
        av = jnp.einsum('bhl,bhld->bhd', p, v)
        av = av.reshape(b, H * dv) if concatenate else av.mean(axis=1)
        return node @ wp.T + bp + av

    def fwd(node, neigh_raw, degs, w):
        mask = jnp.arange(L, dtype=jnp.int32)[None, :] < degs[:, None]
        neigh = jnp.where(mask[:, :, None], neigh_raw, jnp.float32(0.0))
        x = attn(node, neigh, mask, w['wq0'], w['bq0'], w['wv0'], w['bv0'],
                 w['wp0'], w['bp0'], True)
        x = jax.nn.relu(x)
        x = attn(x, neigh, mask, w['wq1'], w['bq1'], w['wv1'], w['bv1'],
                 w['wp1'], w['bp1'], False)
        return x

    pf = jax.pmap(fwd, in_axes=(0, 0, 0, None), devices=devs)

    S = B // NCORES
    node_s = node_embeds.reshape(NCORES, S, D)
    neigh_s = neighbor_embeds.reshape(NCORES, S, L, D)
    deg_s = node_degrees.astype(np.int32).reshape(NCORES, S)
    out = pf(node_s, neigh_s, deg_s, weights)
    return np.asarray(out).reshape(B, D).astype(np.float32)


def kernel(**inputs):
    node_embeds = np.asarray(inputs['node_embeds'], dtype=np.float32)
    neighbor_embeds = np.asarray(inputs['neighbor_embeds'], dtype=np.float32)
    node_degrees = np.asarray(inputs['node_degrees'])
    w = {k: np.asarray(inputs[k], dtype=np.float32) for k in
         ('wq0', 'bq0', 'wv0', 'bv0', 'wp0', 'bp0',
          'wq1', 'bq1', 'wv1', 'bv1', 'wp1', 'bp1')}
    try:
        return _forward_trn(node_embeds, neighbor_embeds, node_degrees, w)
    except Exception:
        return _forward_np(node_embeds, neighbor_embeds, node_degrees,
                           w['wq0'], w['bq0'], w['wv0'], w['bv0'], w['wp0'], w['bp0'],
                           w['wq1'], w['bq1'], w['wv1'], w['bv1'], w['wp1'], w['bp1'])
